# revision 3
# baseline (speedup 1.0000x reference)
"""MoE router (MixLora) Trainium2 kernel.

Computes, for hidden_states [16384, 4096] f32 and gate_weight [8, 4096] f32:
  logits = hidden @ gate.T            -> top-2 (values renormalized) + indices
  aux_loss = E * sum_i f_i * p_i      (load-balance loss)

Sharding: data-parallel over tokens across 8 NeuronCores (2048 tokens/core).
The gate weight is replicated. Aux-loss partials (per-expert softmax-prob sums
and top-2 counts) are computed per core and reduced on host during the gather.

Device strategy per core:
  - hidden shard is staged host-side as transposed fp16 hi/lo pair (ht ~ hi+lo)
    so the contraction dim D lands on SBUF partitions. Same HBM bytes as fp32.
  - logits.T accumulated in PSUM via 3 matmul passes (hi*ghi, hi*glo, lo*ghi)
    with the tiny packed gate as the stationary operand -> fp32-class accuracy
    at bf16-rate PE throughput.
  - logits.T [8, T] tiles are PE-transposed (identity trick) to [128, 8] so
    softmax / top-2 run along the free dim: DVE sort (max/max_index) gives the
    full descending order of the 8 experts; weights via w1 = 1/(1+exp(l2-l1)).
"""

import sys

if "/opt/trn_rl_repo" not in sys.path:
    sys.path.insert(0, "/opt/trn_rl_repo")

from contextlib import ExitStack

import numpy as np

import concourse.bass as bass
import concourse.tile as tile
from concourse import bacc, mybir
from concourse.bass_utils import run_bass_kernel_spmd

N_CORES = 8
T, D, E = 16384, 4096, 8
TPC = T // N_CORES            # tokens per core: 2048
KC = D // 128                 # contraction chunks: 32
NG = TPC // 512               # 512-token groups per core: 4
NT = TPC // 128               # 128-token tiles per core: 16
KJ = 2                        # k-chunks fetched per DMA (1 MiB per transfer)

f32 = mybir.dt.float32
f16 = mybir.dt.float16
i32 = mybir.dt.int32
u32 = mybir.dt.uint32
Alu = mybir.AluOpType
Act = mybir.ActivationFunctionType


def _body(ctx, tc, hth, htl, gth, gtl, iden, rw, xi, pacc_o, cacc_o):
    nc = tc.nc

    const = ctx.enter_context(tc.tile_pool(name="const", bufs=1))
    hpool = ctx.enter_context(tc.tile_pool(name="hbuf", bufs=3))
    psmm = ctx.enter_context(tc.tile_pool(name="psmm", bufs=1, space="PSUM"))
    pstp = ctx.enter_context(tc.tile_pool(name="pstp", bufs=2, space="PSUM"))
    work = ctx.enter_context(tc.tile_pool(name="work", bufs=2))
    accp = ctx.enter_context(tc.tile_pool(name="accp", bufs=1))

    gth_sb = const.tile([128, KC * E], f16)
    nc.sync.dma_start(gth_sb[:], gth[:, :])
    gtl_sb = const.tile([128, KC * E], f16)
    nc.sync.dma_start(gtl_sb[:], gtl[:, :])
    iden_sb = const.tile([8, 8], f32)
    nc.sync.dma_start(iden_sb[:], iden[:, :])

    pacc = accp.tile([128, E], f32)
    nc.vector.memset(pacc[:], 0.0)
    cacc = accp.tile([128, E], f32)
    nc.vector.memset(cacc[:], 0.0)
    w_out = accp.tile([128, 2 * NT], f32)
    i_out = accp.tile([128, 2 * NT], i32)

    # ---- logits.T accumulation: psum_g[e, t] = sum_d g[e, d] * h[t, d] ----
    psum_g = [psmm.tile([E, 512], f32, name=f"ps{g}") for g in range(NG)]
    hi_dram = hth.rearrange("(j p) t -> p j t", p=128)
    lo_dram = htl.rearrange("(j p) t -> p j t", p=128)

    for kj in range(KC // KJ):
        hh = hpool.tile([128, KJ, TPC], f16, tag="hh")
        nc.sync.dma_start(hh[:], hi_dram[:, kj * KJ:(kj + 1) * KJ, :])
        hl = hpool.tile([128, KJ, TPC], f16, tag="hl")
        nc.sync.dma_start(hl[:], lo_dram[:, kj * KJ:(kj + 1) * KJ, :])
        for j in range(KJ):
            k = kj * KJ + j
            wsl = slice(k * E, (k + 1) * E)
            for g in range(NG):
                tsl = slice(g * 512, (g + 1) * 512)
                nc.tensor.matmul(psum_g[g][:], gth_sb[:, wsl], hh[:, j, tsl],
                                 start=(k == 0), stop=False)
            for g in range(NG):
                tsl = slice(g * 512, (g + 1) * 512)
                nc.tensor.matmul(psum_g[g][:], gtl_sb[:, wsl], hh[:, j, tsl],
                                 start=False, stop=False)
            for g in range(NG):
                tsl = slice(g * 512, (g + 1) * 512)
                nc.tensor.matmul(psum_g[g][:], gth_sb[:, wsl], hl[:, j, tsl],
                                 start=False, stop=(k == KC - 1))

    # ---- per 512-token group: transpose to [tok, E] and run routing math ----
    for g in range(NG):
        lg = work.tile([E, 512], f32, tag="lg")
        nc.vector.tensor_copy(lg[:], psum_g[g][:])
        for c in range(4):
            t = g * 4 + c
            ptp = pstp.tile([128, E], f32, tag="tp")
            nc.tensor.transpose(ptp[:], lg[:, c * 128:(c + 1) * 128], iden_sb[:])
            lt = work.tile([128, E], f32, tag="lt")
            nc.vector.tensor_copy(lt[:], ptp[:])

            srt = work.tile([128, E], f32, tag="srt")
            nc.vector.max(srt[:], lt[:])
            sidx = work.tile([128, E], u32, tag="sidx")
            nc.vector.max_index(sidx[:], srt[:], lt[:])

            # aux partials: pacc += softmax(lt); cacc += onehot(top1)+onehot(top2)
            prob = work.tile([128, E], f32, tag="prob")
            z = work.tile([128, 1], f32, tag="z")
            nc.scalar.activation(prob[:], lt[:], Act.Exp, accum_out=z[:])
            rz = work.tile([128, 1], f32, tag="rz")
            nc.vector.reciprocal(rz[:], z[:])
            nc.vector.scalar_tensor_tensor(pacc[:], prob[:], rz[:, 0:1], pacc[:],
                                           Alu.mult, Alu.add)
            nc.vector.scalar_tensor_tensor(cacc[:], lt[:], srt[:, 0:1], cacc[:],
                                           Alu.is_equal, Alu.add)
            nc.vector.scalar_tensor_tensor(cacc[:], lt[:], srt[:, 1:2], cacc[:],
                                           Alu.is_equal, Alu.add)

            # renormalized top-2 weights: w1 = 1/(1+e2), w2 = e2/(1+e2)
            d2 = work.tile([128, 1], f32, tag="d2")
            nc.vector.tensor_tensor(d2[:], srt[:, 1:2], srt[:, 0:1], Alu.subtract)
            e2 = work.tile([128, 1], f32, tag="e2")
            nc.scalar.activation(e2[:], d2[:], Act.Exp)
            t1 = work.tile([128, 1], f32, tag="t1")
            nc.vector.tensor_scalar_add(t1[:], e2[:], 1.0)
            w1 = w_out[:, 2 * t:2 * t + 1]
            nc.vector.reciprocal(w1, t1[:])
            nc.vector.tensor_tensor(w_out[:, 2 * t + 1:2 * t + 2], e2[:], w1,
                                    Alu.mult)
            nc.vector.tensor_copy(i_out[:, 2 * t:2 * t + 2], sidx[:, 0:2])

    nc.sync.dma_start(rw.rearrange("(t p) c -> p t c", p=128),
                      w_out[:].rearrange("p (t c) -> p t c", c=2))
    nc.sync.dma_start(xi.rearrange("(t p) c -> p t c", p=128),
                      i_out[:].rearrange("p (t c) -> p t c", c=2))
    nc.sync.dma_start(pacc_o[:, :], pacc[:])
    nc.sync.dma_start(cacc_o[:, :], cacc[:])


def _build():
    nc = bacc.Bacc("TRN2", target_bir_lowering=False, debug=False,
                   num_devices=N_CORES)
    hth = nc.dram_tensor("hth", [D, TPC], f16, kind="ExternalInput").ap()
    htl = nc.dram_tensor("htl", [D, TPC], f16, kind="ExternalInput").ap()
    gth = nc.dram_tensor("gth", [128, KC * E], f16, kind="ExternalInput").ap()
    gtl = nc.dram_tensor("gtl", [128, KC * E], f16, kind="ExternalInput").ap()
    iden = nc.dram_tensor("iden", [8, 8], f32, kind="ExternalInput").ap()
    rw = nc.dram_tensor("rw", [TPC, 2], f32, kind="ExternalOutput").ap()
    xi = nc.dram_tensor("xi", [TPC, 2], i32, kind="ExternalOutput").ap()
    pacc_o = nc.dram_tensor("pacc", [128, E], f32, kind="ExternalOutput").ap()
    cacc_o = nc.dram_tensor("cacc", [128, E], f32, kind="ExternalOutput").ap()

    with tile.TileContext(nc) as tc:
        with ExitStack() as ctx:
            _body(ctx, tc, hth, htl, gth, gtl, iden, rw, xi, pacc_o, cacc_o)
    nc.compile()
    return nc


_NC_CACHE = {}


def _get_nc():
    if "nc" not in _NC_CACHE:
        _NC_CACHE["nc"] = _build()
    return _NC_CACHE["nc"]


def _ensure_ntff_hook():
    """Register the axon NTFF profiling hook (the image's antenv lacks
    axon_hooks, so the boot-time registration degraded silently)."""
    import types

    if "antenv.axon_hooks" in sys.modules:
        return
    mod = types.ModuleType("antenv.axon_hooks")
    state = {"hook": None}
    mod.set_axon_ntff_profile_hook = lambda h: state.__setitem__("hook", h)
    mod.get_axon_ntff_profile_hook = lambda: state["hook"]
    sys.modules["antenv.axon_hooks"] = mod
    import antenv

    antenv.axon_hooks = mod
    try:
        from trn_agent_boot.trn_boot import _ntff_profile_via_ctypes

        hook = _ntff_profile_via_ctypes("/opt/axon/libaxon_pjrt.so")
        if hook is not None:
            mod.set_axon_ntff_profile_hook(hook)
    except Exception:
        pass
    # keep profiling artifacts local — no bucket in this container
    import concourse.bass_utils as bu

    bu.upload_artifacts = lambda tmpdir: tmpdir


def _pack_gate(gate_weight):
    """gate [E, D] f32 -> (hi, lo) packed [128, KC*E] fp16 with
    packed[p, k*E+e] = g(e, 128k+p)."""
    gt = np.ascontiguousarray(gate_weight.T.astype(np.float32))     # [D, E]
    hi = gt.astype(np.float16)
    lo = (gt - hi.astype(np.float32)).astype(np.float16)

    def pack(x):
        return np.ascontiguousarray(
            x.reshape(KC, 128, E).transpose(1, 0, 2).reshape(128, KC * E))

    return pack(hi), pack(lo)


def _prep_inputs(hidden_states, gate_weight):
    hidden_states = np.asarray(hidden_states, dtype=np.float32)
    gate_weight = np.asarray(gate_weight, dtype=np.float32)
    ghi, glo = _pack_gate(gate_weight)
    iden = np.eye(8, dtype=np.float32)
    in_maps = []
    for c in range(N_CORES):
        sh = np.ascontiguousarray(
            hidden_states[c * TPC:(c + 1) * TPC].T)                 # [D, TPC]
        hi = sh.astype(np.float16)
        lo = (sh - hi.astype(np.float32)).astype(np.float16)
        in_maps.append({"hth": hi, "htl": lo, "gth": ghi, "gtl": glo,
                        "iden": iden})
    return in_maps


def _finalize(results):
    rw = np.concatenate([r["rw"] for r in results], axis=0)
    xi = np.concatenate([r["xi"] for r in results], axis=0).astype(np.int32)
    psum = np.zeros(E, np.float64)
    csum = np.zeros(E, np.float64)
    for r in results:
        psum += r["pacc"].astype(np.float64).sum(axis=0)
        csum += r["cacc"].astype(np.float64).sum(axis=0)
    p_i = (psum / T).astype(np.float32)
    f_i = (csum / T).astype(np.float32)
    aux = np.float32(E) * np.dot(f_i, p_i)
    return rw.astype(np.float32), xi, np.float32(aux)


def run(hidden_states, gate_weight, trace=False, tmpdir=None):
    nc = _get_nc()
    if trace:
        _ensure_ntff_hook()
    in_maps = _prep_inputs(hidden_states, gate_weight)
    res = run_bass_kernel_spmd(nc, in_maps, list(range(N_CORES)), trace=trace,
                               tmpdir=tmpdir)
    return _finalize(res.results), res


def kernel(hidden_states, gate_weight):
    (rw, xi, aux), _ = run(hidden_states, gate_weight, trace=False)
    return rw, xi, aux


# revision 13
# speedup vs baseline: 1.1171x; 1.1171x over previous
"""MoE router (MixLora) Trainium2 kernel.

Computes, for hidden_states [16384, 4096] f32 and gate_weight [8, 4096] f32:
  logits = hidden @ gate.T            -> top-2 (values renormalized) + indices
  aux_loss = E * sum_i f_i * p_i      (load-balance loss)

Sharding: data-parallel over tokens across 8 NeuronCores (2048 tokens/core).
The gate weight is replicated. Aux-loss partials (per-expert softmax-prob sums
and top-2 counts) are computed per core and reduced on host during the gather.

Device strategy per core:
  - hidden shard is staged host-side as transposed fp16 hi/lo pair (ht ~ hi+lo)
    so the contraction dim D lands on SBUF partitions. Same HBM bytes as fp32.
  - logits.T accumulated in PSUM via 3 matmul passes (hi*ghi, hi*glo, lo*ghi)
    with the tiny packed gate as the stationary operand -> fp32-class accuracy
    at bf16-rate PE throughput.
  - logits.T [8, T] tiles are PE-transposed (identity trick) to [128, 8] so
    softmax / top-2 run along the free dim: DVE sort (max/max_index) gives the
    full descending order of the 8 experts; weights via w1 = 1/(1+exp(l2-l1)).
"""

import sys

if "/opt/trn_rl_repo" not in sys.path:
    sys.path.insert(0, "/opt/trn_rl_repo")

from contextlib import ExitStack

import numpy as np

import concourse.bass as bass
import concourse.tile as tile
from concourse import bacc, mybir
from concourse.bass_utils import run_bass_kernel_spmd

N_CORES = 8
T, D, E = 16384, 4096, 8
TPC = T // N_CORES            # tokens per core: 2048
KC = D // 128                 # contraction chunks: 32
NT = TPC // 128               # 128-token tiles per core: 16
NP = 2                        # token phases (epilogue overlaps next phase's DMA)
TPP = TPC // NP               # tokens per phase: 1024
GPP = TPP // 512              # 512-token groups per phase: 2
KJ = 4                        # k-chunks fetched per DMA (1 MiB per transfer)

f32 = mybir.dt.float32
f16 = mybir.dt.float16
i32 = mybir.dt.int32
u32 = mybir.dt.uint32
Alu = mybir.AluOpType
Act = mybir.ActivationFunctionType


def _body(ctx, tc, hth, htl, gph, iden, rw, xi, pacc_o, cacc_o):
    nc = tc.nc

    const = ctx.enter_context(tc.tile_pool(name="const", bufs=1))
    hpool = ctx.enter_context(tc.tile_pool(name="hbuf", bufs=4))
    pstp = ctx.enter_context(tc.tile_pool(name="pstp", bufs=2, space="PSUM"))
    work = ctx.enter_context(tc.tile_pool(name="work", bufs=2))
    accp = ctx.enter_context(tc.tile_pool(name="accp", bufs=1))

    gph_sb = const.tile([128, KC * 2 * E], f16)
    nc.sync.dma_start(gph_sb[:], gph[:, :])
    iden_sb = const.tile([2 * E, 2 * E], f32)
    nc.sync.dma_start(iden_sb[:], iden[:, :])

    pacc = accp.tile([128, E], f32)
    nc.vector.memset(pacc[:], 0.0)
    cacc = accp.tile([128, E], f32)
    nc.vector.memset(cacc[:], 0.0)
    w_out = accp.tile([128, 2 * NT], f32)
    i_out = accp.tile([128, 2 * NT], i32)

    # ---- logits.T: ps[0:8]  = (ghi . hh) + (ghi . hl)   (hl pass accumulates)
    #               ps[8:16] = (glo . hh)                 -> combined post-transpose
    psmm_ps = ctx.enter_context(tc.tile_pool(name="psbank", bufs=2, space="PSUM"))
    hi_dram = hth.rearrange("(j p) t -> p j t", p=128)
    lo_dram = htl.rearrange("(j p) t -> p j t", p=128)

    for ph in range(NP):
        tph = slice(ph * TPP, (ph + 1) * TPP)
        psum_g = [psmm_ps.tile([2 * E, 512], f32, name=f"ps{g}", tag=f"ps{g}")
                  for g in range(GPP)]
        for kj in range(KC // KJ):
            hh = hpool.tile([128, KJ, TPP], f16, tag="hh")
            nc.sync.dma_start(hh[:], hi_dram[:, kj * KJ:(kj + 1) * KJ, tph])
            hl = hpool.tile([128, KJ, TPP], f16, tag="hl")
            nc.sync.dma_start(hl[:], lo_dram[:, kj * KJ:(kj + 1) * KJ, tph])
            for j in range(KJ):
                k = kj * KJ + j
                last = k == KC - 1
                wsl = slice(k * 2 * E, (k + 1) * 2 * E)
                whsl = slice(k * 2 * E, k * 2 * E + E)

                def mm_hh(g, stop):
                    tsl = slice(g * 512, (g + 1) * 512)
                    nc.tensor.matmul(psum_g[g][:, :], gph_sb[:, wsl],
                                     hh[:, j, tsl], start=(k == 0), stop=stop)

                def mm_hl(g):
                    tsl = slice(g * 512, (g + 1) * 512)
                    nc.tensor.matmul(psum_g[g][0:E, :], gph_sb[:, whsl],
                                     hl[:, j, tsl], start=False, stop=False)

                if last:
                    # hl pass first so the final hh pass can close the
                    # accumulation group over the full [0:2E] region
                    for g in range(GPP):
                        mm_hl(g)
                    for g in range(GPP):
                        mm_hh(g, stop=True)
                else:
                    for g in range(GPP):
                        mm_hh(g, stop=False)
                    for g in range(GPP):
                        mm_hl(g)

        # ---- per 512-token group: transpose to [tok, 2E], combine halves,
        #      then routing math along the free dim ----
        for g in range(GPP):
            lg = work.tile([2 * E, 512], f32, tag="lg")
            nc.vector.tensor_copy(lg[:], psum_g[g][:])
            for c in range(4):
                t = (ph * GPP + g) * 4 + c
                ptp = pstp.tile([128, 2 * E], f32, tag="tp")
                nc.tensor.transpose(ptp[:], lg[:, c * 128:(c + 1) * 128],
                                    iden_sb[:])
                ltw = work.tile([128, 2 * E], f32, tag="ltw")
                nc.vector.tensor_copy(ltw[:], ptp[:])
                lt = work.tile([128, E], f32, tag="lt")
                nc.vector.tensor_tensor(lt[:], ltw[:, 0:E], ltw[:, E:2 * E],
                                        Alu.add)

                srt = work.tile([128, E], f32, tag="srt")
                nc.vector.max(srt[:], lt[:])
                sidx = work.tile([128, E], u32, tag="sidx")
                nc.vector.max_index(sidx[:], srt[:], lt[:])

                # aux partials: pacc += softmax(lt); cacc += top-2 one-hots
                prob = work.tile([128, E], f32, tag="prob")
                z = work.tile([128, 1], f32, tag="z")
                nc.scalar.activation(prob[:], lt[:], Act.Exp, accum_out=z[:])
                rz = work.tile([128, 1], f32, tag="rz")
                nc.vector.reciprocal(rz[:], z[:])
                nc.vector.scalar_tensor_tensor(pacc[:], prob[:], rz[:, 0:1],
                                               pacc[:], Alu.mult, Alu.add)
                nc.vector.scalar_tensor_tensor(cacc[:], lt[:], srt[:, 0:1],
                                               cacc[:], Alu.is_equal, Alu.add)
                nc.vector.scalar_tensor_tensor(cacc[:], lt[:], srt[:, 1:2],
                                               cacc[:], Alu.is_equal, Alu.add)

                # renormalized top-2 weights: w1 = 1/(1+e2), w2 = e2/(1+e2)
                d2 = work.tile([128, 1], f32, tag="d2")
                nc.vector.tensor_tensor(d2[:], srt[:, 1:2], srt[:, 0:1],
                                        Alu.subtract)
                e2 = work.tile([128, 1], f32, tag="e2")
                nc.scalar.activation(e2[:], d2[:], Act.Exp)
                t1 = work.tile([128, 1], f32, tag="t1")
                nc.vector.tensor_scalar_add(t1[:], e2[:], 1.0)
                w1 = w_out[:, 2 * t:2 * t + 1]
                nc.vector.reciprocal(w1, t1[:])
                nc.vector.tensor_tensor(w_out[:, 2 * t + 1:2 * t + 2], e2[:],
                                        w1, Alu.mult)
                nc.vector.tensor_copy(i_out[:, 2 * t:2 * t + 2], sidx[:, 0:2])

    nc.sync.dma_start(rw.rearrange("(t p) c -> p t c", p=128),
                      w_out[:].rearrange("p (t c) -> p t c", c=2))
    nc.sync.dma_start(xi.rearrange("(t p) c -> p t c", p=128),
                      i_out[:].rearrange("p (t c) -> p t c", c=2))
    nc.sync.dma_start(pacc_o[:, :], pacc[:])
    nc.sync.dma_start(cacc_o[:, :], cacc[:])


def _build():
    nc = bacc.Bacc("TRN2", target_bir_lowering=False, debug=False,
                   num_devices=N_CORES)
    hth = nc.dram_tensor("hth", [D, TPC], f16, kind="ExternalInput").ap()
    htl = nc.dram_tensor("htl", [D, TPC], f16, kind="ExternalInput").ap()
    gph = nc.dram_tensor("gph", [128, KC * 2 * E], f16,
                         kind="ExternalInput").ap()
    iden = nc.dram_tensor("iden", [2 * E, 2 * E], f32,
                          kind="ExternalInput").ap()
    rw = nc.dram_tensor("rw", [TPC, 2], f32, kind="ExternalOutput").ap()
    xi = nc.dram_tensor("xi", [TPC, 2], i32, kind="ExternalOutput").ap()
    pacc_o = nc.dram_tensor("pacc", [128, E], f32, kind="ExternalOutput").ap()
    cacc_o = nc.dram_tensor("cacc", [128, E], f32, kind="ExternalOutput").ap()

    with tile.TileContext(nc) as tc:
        with ExitStack() as ctx:
            _body(ctx, tc, hth, htl, gph, iden, rw, xi, pacc_o, cacc_o)
    nc.compile()
    return nc


_NC_CACHE = {}


def _get_nc():
    if "nc" not in _NC_CACHE:
        _NC_CACHE["nc"] = _build()
    return _NC_CACHE["nc"]


def _ensure_ntff_hook():
    """Register the axon NTFF profiling hook (the image's antenv lacks
    axon_hooks, so the boot-time registration degraded silently)."""
    import types

    if "antenv.axon_hooks" in sys.modules:
        return
    mod = types.ModuleType("antenv.axon_hooks")
    state = {"hook": None}
    mod.set_axon_ntff_profile_hook = lambda h: state.__setitem__("hook", h)
    mod.get_axon_ntff_profile_hook = lambda: state["hook"]
    sys.modules["antenv.axon_hooks"] = mod
    import antenv

    antenv.axon_hooks = mod
    try:
        from trn_agent_boot.trn_boot import _ntff_profile_via_ctypes

        hook = _ntff_profile_via_ctypes("/opt/axon/libaxon_pjrt.so")
        if hook is not None:
            mod.set_axon_ntff_profile_hook(hook)
    except Exception:
        pass
    # keep profiling artifacts local — no bucket in this container
    import concourse.bass_utils as bu

    bu.upload_artifacts = lambda tmpdir: tmpdir


def _pack_gate(gate_weight):
    """gate [E, D] f32 -> packed [128, KC*2E] fp16 with
    packed[p, k*2E + e]     = hi(g)(e, 128k+p)   (e < E)
    packed[p, k*2E + E + e] = lo(g)(e, 128k+p)."""
    gt = np.ascontiguousarray(gate_weight.T.astype(np.float32))     # [D, E]
    hi = gt.astype(np.float16)
    lo = (gt - hi.astype(np.float32)).astype(np.float16)
    both = np.concatenate([hi.reshape(KC, 128, E), lo.reshape(KC, 128, E)],
                          axis=2)                                   # [KC,128,2E]
    return np.ascontiguousarray(
        both.transpose(1, 0, 2).reshape(128, KC * 2 * E))


def _prep_inputs(hidden_states, gate_weight):
    hidden_states = np.asarray(hidden_states, dtype=np.float32)
    gate_weight = np.asarray(gate_weight, dtype=np.float32)
    gph = _pack_gate(gate_weight)
    iden = np.eye(2 * E, dtype=np.float32)
    in_maps = []
    for c in range(N_CORES):
        sh = np.ascontiguousarray(
            hidden_states[c * TPC:(c + 1) * TPC].T)                 # [D, TPC]
        hi = sh.astype(np.float16)
        lo = (sh - hi.astype(np.float32)).astype(np.float16)
        in_maps.append({"hth": hi, "htl": lo, "gph": gph, "iden": iden})
    return in_maps


def _finalize(results):
    rw = np.concatenate([r["rw"] for r in results], axis=0)
    xi = np.concatenate([r["xi"] for r in results], axis=0).astype(np.int32)
    psum = np.zeros(E, np.float64)
    csum = np.zeros(E, np.float64)
    for r in results:
        psum += r["pacc"].astype(np.float64).sum(axis=0)
        csum += r["cacc"].astype(np.float64).sum(axis=0)
    p_i = (psum / T).astype(np.float32)
    f_i = (csum / T).astype(np.float32)
    aux = np.float32(E) * np.dot(f_i, p_i)
    return rw.astype(np.float32), xi, np.float32(aux)


def run(hidden_states, gate_weight, trace=False, tmpdir=None):
    nc = _get_nc()
    if trace:
        _ensure_ntff_hook()
    in_maps = _prep_inputs(hidden_states, gate_weight)
    res = run_bass_kernel_spmd(nc, in_maps, list(range(N_CORES)), trace=trace,
                               tmpdir=tmpdir)
    return _finalize(res.results), res


def kernel(hidden_states, gate_weight):
    (rw, xi, aux), _ = run(hidden_states, gate_weight, trace=False)
    return rw, xi, aux


# revision 16
# speedup vs baseline: 1.1935x; 1.0684x over previous
"""MoE router (MixLora) Trainium2 kernel.

Computes, for hidden_states [16384, 4096] f32 and gate_weight [8, 4096] f32:
  logits = hidden @ gate.T            -> top-2 (values renormalized) + indices
  aux_loss = E * sum_i f_i * p_i      (load-balance loss)

Sharding: data-parallel over tokens across 8 NeuronCores (2048 tokens/core).
The gate weight is replicated. Aux-loss partials (per-expert softmax-prob sums
and top-2 counts) are computed per core and reduced on host during the gather.

Device strategy per core:
  - hidden shard is staged host-side as transposed fp16 hi/lo pair (ht ~ hi+lo)
    so the contraction dim D lands on SBUF partitions. Same HBM bytes as fp32.
  - logits.T accumulated in PSUM via 3 matmul passes (hi*ghi, hi*glo, lo*ghi)
    with the tiny packed gate as the stationary operand -> fp32-class accuracy
    at bf16-rate PE throughput.
  - logits.T [8, T] tiles are PE-transposed (identity trick) to [128, 8] so
    softmax / top-2 run along the free dim: DVE sort (max/max_index) gives the
    full descending order of the 8 experts; weights via w1 = 1/(1+exp(l2-l1)).
"""

import sys

if "/opt/trn_rl_repo" not in sys.path:
    sys.path.insert(0, "/opt/trn_rl_repo")

from contextlib import ExitStack

import numpy as np

import concourse.bass as bass
import concourse.tile as tile
from concourse import bacc, mybir
from concourse.bass_utils import run_bass_kernel_spmd

N_CORES = 8
T, D, E = 16384, 4096, 8
TPC = T // N_CORES            # tokens per core: 2048
KC = D // 128                 # contraction chunks: 32
NT = TPC // 128               # 128-token tiles per core: 16
NP = 2                        # token phases (epilogue overlaps next phase's DMA)
TPP = TPC // NP               # tokens per phase: 1024
GPP = TPP // 512              # 512-token groups per phase: 2
KJ = 4                        # k-chunks fetched per DMA (1 MiB per transfer)

f32 = mybir.dt.float32
f16 = mybir.dt.float16
i32 = mybir.dt.int32
u32 = mybir.dt.uint32
Alu = mybir.AluOpType
Act = mybir.ActivationFunctionType


def _body(ctx, tc, hth, htl, gph, iden, rw, xi, pacc_o, cacc_o):
    nc = tc.nc

    const = ctx.enter_context(tc.tile_pool(name="const", bufs=1))
    hpool = ctx.enter_context(tc.tile_pool(name="hbuf", bufs=6))
    pstp = ctx.enter_context(tc.tile_pool(name="pstp", bufs=2, space="PSUM"))
    work = ctx.enter_context(tc.tile_pool(name="work", bufs=2))
    accp = ctx.enter_context(tc.tile_pool(name="accp", bufs=1))

    gph_sb = const.tile([128, KC * 2 * E], f16)
    nc.sync.dma_start(gph_sb[:], gph[:, :])
    iden_sb = const.tile([2 * E, 2 * E], f32)
    nc.sync.dma_start(iden_sb[:], iden[:, :])

    pacc = accp.tile([128, E], f32)
    nc.vector.memset(pacc[:], 0.0)
    cacc = accp.tile([128, E], f32)
    nc.vector.memset(cacc[:], 0.0)
    w_out = accp.tile([128, 2 * NT], f32)
    i_out = accp.tile([128, 2 * NT], i32)

    # ---- logits.T: ps[0:8]  = (ghi . hh) + (ghi . hl)   (hl pass accumulates)
    #               ps[8:16] = (glo . hh)                 -> combined post-transpose
    psmm_ps = ctx.enter_context(tc.tile_pool(name="psbank", bufs=2, space="PSUM"))
    hi_dram = hth.rearrange("(j p) t -> p j t", p=128)
    lo_dram = htl.rearrange("(j p) t -> p j t", p=128)

    for ph in range(NP):
        tph = slice(ph * TPP, (ph + 1) * TPP)
        psum_g = [psmm_ps.tile([2 * E, 512], f32, name=f"ps{g}", tag=f"ps{g}")
                  for g in range(GPP)]
        for kj in range(KC // KJ):
            hh = hpool.tile([128, KJ, TPP], f16, tag="hh")
            nc.sync.dma_start(hh[:], hi_dram[:, kj * KJ:(kj + 1) * KJ, tph])
            hl = hpool.tile([128, KJ, TPP], f16, tag="hl")
            nc.scalar.dma_start(hl[:], lo_dram[:, kj * KJ:(kj + 1) * KJ, tph])
            for j in range(KJ):
                k = kj * KJ + j
                last = k == KC - 1
                wsl = slice(k * 2 * E, (k + 1) * 2 * E)
                whsl = slice(k * 2 * E, k * 2 * E + E)

                def mm_hh(g, stop):
                    tsl = slice(g * 512, (g + 1) * 512)
                    nc.tensor.matmul(psum_g[g][:, :], gph_sb[:, wsl],
                                     hh[:, j, tsl], start=(k == 0), stop=stop)

                def mm_hl(g):
                    tsl = slice(g * 512, (g + 1) * 512)
                    nc.tensor.matmul(psum_g[g][0:E, :], gph_sb[:, whsl],
                                     hl[:, j, tsl], start=False, stop=False)

                if last:
                    # hl pass first so the final hh pass can close the
                    # accumulation group over the full [0:2E] region
                    for g in range(GPP):
                        mm_hl(g)
                    for g in range(GPP):
                        mm_hh(g, stop=True)
                else:
                    for g in range(GPP):
                        mm_hh(g, stop=False)
                    for g in range(GPP):
                        mm_hl(g)

        # ---- per 512-token group: transpose to [tok, 2E], combine halves,
        #      then routing math along the free dim ----
        for g in range(GPP):
            lg = work.tile([2 * E, 512], f32, tag="lg")
            nc.vector.tensor_copy(lg[:], psum_g[g][:])
            for c in range(4):
                t = (ph * GPP + g) * 4 + c
                ptp = pstp.tile([128, 2 * E], f32, tag="tp")
                nc.tensor.transpose(ptp[:], lg[:, c * 128:(c + 1) * 128],
                                    iden_sb[:])
                ltw = work.tile([128, 2 * E], f32, tag="ltw")
                nc.vector.tensor_copy(ltw[:], ptp[:])
                lt = work.tile([128, E], f32, tag="lt")
                nc.vector.tensor_tensor(lt[:], ltw[:, 0:E], ltw[:, E:2 * E],
                                        Alu.add)

                srt = work.tile([128, E], f32, tag="srt")
                nc.vector.max(srt[:], lt[:])
                sidx = work.tile([128, E], u32, tag="sidx")
                nc.vector.max_index(sidx[:], srt[:], lt[:])

                # aux partials: pacc += softmax(lt); cacc += top-2 one-hots
                prob = work.tile([128, E], f32, tag="prob")
                z = work.tile([128, 1], f32, tag="z")
                nc.scalar.activation(prob[:], lt[:], Act.Exp, accum_out=z[:])
                rz = work.tile([128, 1], f32, tag="rz")
                nc.vector.reciprocal(rz[:], z[:])
                nc.vector.scalar_tensor_tensor(pacc[:], prob[:], rz[:, 0:1],
                                               pacc[:], Alu.mult, Alu.add)
                nc.vector.scalar_tensor_tensor(cacc[:], lt[:], srt[:, 0:1],
                                               cacc[:], Alu.is_equal, Alu.add)
                nc.vector.scalar_tensor_tensor(cacc[:], lt[:], srt[:, 1:2],
                                               cacc[:], Alu.is_equal, Alu.add)

                # renormalized top-2 weights: w1 = 1/(1+e2), w2 = e2/(1+e2)
                d2 = work.tile([128, 1], f32, tag="d2")
                nc.vector.tensor_tensor(d2[:], srt[:, 1:2], srt[:, 0:1],
                                        Alu.subtract)
                e2 = work.tile([128, 1], f32, tag="e2")
                nc.scalar.activation(e2[:], d2[:], Act.Exp)
                t1 = work.tile([128, 1], f32, tag="t1")
                nc.vector.tensor_scalar_add(t1[:], e2[:], 1.0)
                w1 = w_out[:, 2 * t:2 * t + 1]
                nc.vector.reciprocal(w1, t1[:])
                nc.vector.tensor_tensor(w_out[:, 2 * t + 1:2 * t + 2], e2[:],
                                        w1, Alu.mult)
                nc.vector.tensor_copy(i_out[:, 2 * t:2 * t + 2], sidx[:, 0:2])

        # flush this phase's outputs while the next phase streams
        ntp = NT // NP
        psl = slice(ph * ntp, (ph + 1) * ntp)
        nc.sync.dma_start(
            rw.rearrange("(t p) c -> p t c", p=128)[:, psl, :],
            w_out[:].rearrange("p (t c) -> p t c", c=2)[:, psl, :])
        nc.sync.dma_start(
            xi.rearrange("(t p) c -> p t c", p=128)[:, psl, :],
            i_out[:].rearrange("p (t c) -> p t c", c=2)[:, psl, :])

    nc.sync.dma_start(pacc_o[:, :], pacc[:])
    nc.sync.dma_start(cacc_o[:, :], cacc[:])


def _build():
    nc = bacc.Bacc("TRN2", target_bir_lowering=False, debug=False,
                   num_devices=N_CORES)
    hth = nc.dram_tensor("hth", [D, TPC], f16, kind="ExternalInput").ap()
    htl = nc.dram_tensor("htl", [D, TPC], f16, kind="ExternalInput").ap()
    gph = nc.dram_tensor("gph", [128, KC * 2 * E], f16,
                         kind="ExternalInput").ap()
    iden = nc.dram_tensor("iden", [2 * E, 2 * E], f32,
                          kind="ExternalInput").ap()
    rw = nc.dram_tensor("rw", [TPC, 2], f32, kind="ExternalOutput").ap()
    xi = nc.dram_tensor("xi", [TPC, 2], i32, kind="ExternalOutput").ap()
    pacc_o = nc.dram_tensor("pacc", [128, E], f32, kind="ExternalOutput").ap()
    cacc_o = nc.dram_tensor("cacc", [128, E], f32, kind="ExternalOutput").ap()

    with tile.TileContext(nc) as tc:
        with ExitStack() as ctx:
            _body(ctx, tc, hth, htl, gph, iden, rw, xi, pacc_o, cacc_o)
    nc.compile()
    return nc


_NC_CACHE = {}


def _get_nc():
    if "nc" not in _NC_CACHE:
        _NC_CACHE["nc"] = _build()
    return _NC_CACHE["nc"]


def _ensure_ntff_hook():
    """Register the axon NTFF profiling hook (the image's antenv lacks
    axon_hooks, so the boot-time registration degraded silently)."""
    import types

    if "antenv.axon_hooks" in sys.modules:
        return
    mod = types.ModuleType("antenv.axon_hooks")
    state = {"hook": None}
    mod.set_axon_ntff_profile_hook = lambda h: state.__setitem__("hook", h)
    mod.get_axon_ntff_profile_hook = lambda: state["hook"]
    sys.modules["antenv.axon_hooks"] = mod
    import antenv

    antenv.axon_hooks = mod
    try:
        from trn_agent_boot.trn_boot import _ntff_profile_via_ctypes

        hook = _ntff_profile_via_ctypes("/opt/axon/libaxon_pjrt.so")
        if hook is not None:
            mod.set_axon_ntff_profile_hook(hook)
    except Exception:
        pass
    # keep profiling artifacts local — no bucket in this container
    import concourse.bass_utils as bu

    bu.upload_artifacts = lambda tmpdir: tmpdir


def _pack_gate(gate_weight):
    """gate [E, D] f32 -> packed [128, KC*2E] fp16 with
    packed[p, k*2E + e]     = hi(g)(e, 128k+p)   (e < E)
    packed[p, k*2E + E + e] = lo(g)(e, 128k+p)."""
    gt = np.ascontiguousarray(gate_weight.T.astype(np.float32))     # [D, E]
    hi = gt.astype(np.float16)
    lo = (gt - hi.astype(np.float32)).astype(np.float16)
    both = np.concatenate([hi.reshape(KC, 128, E), lo.reshape(KC, 128, E)],
                          axis=2)                                   # [KC,128,2E]
    return np.ascontiguousarray(
        both.transpose(1, 0, 2).reshape(128, KC * 2 * E))


def _prep_inputs(hidden_states, gate_weight):
    hidden_states = np.asarray(hidden_states, dtype=np.float32)
    gate_weight = np.asarray(gate_weight, dtype=np.float32)
    gph = _pack_gate(gate_weight)
    iden = np.eye(2 * E, dtype=np.float32)
    in_maps = []
    for c in range(N_CORES):
        sh = np.ascontiguousarray(
            hidden_states[c * TPC:(c + 1) * TPC].T)                 # [D, TPC]
        hi = sh.astype(np.float16)
        lo = (sh - hi.astype(np.float32)).astype(np.float16)
        in_maps.append({"hth": hi, "htl": lo, "gph": gph, "iden": iden})
    return in_maps


def _finalize(results):
    rw = np.concatenate([r["rw"] for r in results], axis=0)
    xi = np.concatenate([r["xi"] for r in results], axis=0).astype(np.int32)
    psum = np.zeros(E, np.float64)
    csum = np.zeros(E, np.float64)
    for r in results:
        psum += r["pacc"].astype(np.float64).sum(axis=0)
        csum += r["cacc"].astype(np.float64).sum(axis=0)
    p_i = (psum / T).astype(np.float32)
    f_i = (csum / T).astype(np.float32)
    aux = np.float32(E) * np.dot(f_i, p_i)
    return rw.astype(np.float32), xi, np.float32(aux)


def run(hidden_states, gate_weight, trace=False, tmpdir=None):
    nc = _get_nc()
    if trace:
        _ensure_ntff_hook()
    in_maps = _prep_inputs(hidden_states, gate_weight)
    res = run_bass_kernel_spmd(nc, in_maps, list(range(N_CORES)), trace=trace,
                               tmpdir=tmpdir)
    return _finalize(res.results), res


def kernel(hidden_states, gate_weight):
    (rw, xi, aux), _ = run(hidden_states, gate_weight, trace=False)
    return rw, xi, aux


# revision 21
# speedup vs baseline: 1.3053x; 1.0937x over previous
"""MoE router (MixLora) Trainium2 kernel.

Computes, for hidden_states [16384, 4096] f32 and gate_weight [8, 4096] f32:
  logits = hidden @ gate.T            -> top-2 (values renormalized) + indices
  aux_loss = E * sum_i f_i * p_i      (load-balance loss)

Sharding: data-parallel over tokens across 8 NeuronCores (2048 tokens/core).
The gate weight is replicated. Aux-loss partials (per-expert softmax-prob sums
and top-2 counts) are computed per core and reduced on host during the gather.

Device strategy per core:
  - hidden shard is staged host-side as transposed fp16 hi/lo pair (ht ~ hi+lo)
    so the contraction dim D lands on SBUF partitions. Same HBM bytes as fp32.
  - logits.T accumulated in PSUM via 3 matmul passes (hi*ghi, hi*glo, lo*ghi)
    with the tiny packed gate as the stationary operand -> fp32-class accuracy
    at bf16-rate PE throughput.
  - logits.T [8, T] tiles are PE-transposed (identity trick) to [128, 8] so
    softmax / top-2 run along the free dim: DVE sort (max/max_index) gives the
    full descending order of the 8 experts; weights via w1 = 1/(1+exp(l2-l1)).
"""

import sys

if "/opt/trn_rl_repo" not in sys.path:
    sys.path.insert(0, "/opt/trn_rl_repo")

from contextlib import ExitStack

import numpy as np

import concourse.bass as bass
import concourse.tile as tile
from concourse import bacc, mybir
from concourse.bass_utils import run_bass_kernel_spmd

N_CORES = 8
T, D, E = 16384, 4096, 8
TPC = T // N_CORES            # tokens per core: 2048
KC = D // 128                 # contraction chunks: 32
NT = TPC // 128               # 128-token tiles per core: 16
# Token phases: uneven split so the final phase (whose matmuls + epilogue
# serialize after the DMA stream ends) is small. Per-phase KJ targets ~1 MiB
# DMA transfers (tile = [128, KJ, TPP] fp16).
PHASES = [(1536, 2), (512, 8)]        # (tokens, k-chunks per DMA)

f32 = mybir.dt.float32
f16 = mybir.dt.float16
i32 = mybir.dt.int32
u32 = mybir.dt.uint32
Alu = mybir.AluOpType
Act = mybir.ActivationFunctionType


def _body(ctx, tc, hth, htl, gph, iden, rw, xi, pacc_o, cacc_o):
    nc = tc.nc

    const = ctx.enter_context(tc.tile_pool(name="const", bufs=1))
    hpool = ctx.enter_context(tc.tile_pool(name="hbuf", bufs=6))
    pstp = ctx.enter_context(tc.tile_pool(name="pstp", bufs=2, space="PSUM"))
    work = ctx.enter_context(tc.tile_pool(name="work", bufs=2))
    accp = ctx.enter_context(tc.tile_pool(name="accp", bufs=1))

    # const loads via SWDGE so the HWDGE rings start with the big streams
    gph_sb = const.tile([128, KC * 2 * E], f16)
    nc.gpsimd.dma_start(gph_sb[:], gph[:, :])
    iden_sb = const.tile([2 * E, 2 * E], f32)
    nc.gpsimd.dma_start(iden_sb[:], iden[:, :])

    pacc = accp.tile([128, E], f32)
    nc.vector.memset(pacc[:], 0.0)
    cacc = accp.tile([128, E], f32)
    nc.vector.memset(cacc[:], 0.0)
    w_out = accp.tile([128, 2 * NT], f32)
    i_out = accp.tile([128, 2 * NT], i32)

    # ---- logits.T: ps[0:8]  = (ghi . hh) + (ghi . hl)   (hl pass accumulates)
    #               ps[8:16] = (glo . hh)                 -> combined post-transpose
    psmm_ps = ctx.enter_context(tc.tile_pool(name="psbank", bufs=2, space="PSUM"))
    hi_dram = hth.rearrange("(j p) t -> p j t", p=128)
    lo_dram = htl.rearrange("(j p) t -> p j t", p=128)

    tok0 = 0
    tile0 = 0
    for ph, (TPP, KJ) in enumerate(PHASES):
        GPP = TPP // 512
        tph = slice(tok0, tok0 + TPP)
        psum_g = [psmm_ps.tile([2 * E, 512], f32, name=f"ps{g}", tag=f"ps{g}")
                  for g in range(GPP)]
        for kj in range(KC // KJ):
            hh = hpool.tile([128, KJ, TPP], f16, tag="hh")
            nc.sync.dma_start(hh[:], hi_dram[:, kj * KJ:(kj + 1) * KJ, tph])
            hl = hpool.tile([128, KJ, TPP], f16, tag="hl")
            nc.scalar.dma_start(hl[:], lo_dram[:, kj * KJ:(kj + 1) * KJ, tph])
            for j in range(KJ):
                k = kj * KJ + j
                last = k == KC - 1
                wsl = slice(k * 2 * E, (k + 1) * 2 * E)
                whsl = slice(k * 2 * E, k * 2 * E + E)

                def mm_hh(g, stop):
                    tsl = slice(g * 512, (g + 1) * 512)
                    nc.tensor.matmul(psum_g[g][:, :], gph_sb[:, wsl],
                                     hh[:, j, tsl], start=(k == 0), stop=stop)

                def mm_hl(g):
                    tsl = slice(g * 512, (g + 1) * 512)
                    nc.tensor.matmul(psum_g[g][0:E, :], gph_sb[:, whsl],
                                     hl[:, j, tsl], start=False, stop=False)

                if last:
                    # hl pass first so the final hh pass can close the
                    # accumulation group over the full [0:2E] region
                    for g in range(GPP):
                        mm_hl(g)
                    for g in range(GPP):
                        mm_hh(g, stop=True)
                else:
                    for g in range(GPP):
                        mm_hh(g, stop=False)
                    for g in range(GPP):
                        mm_hl(g)

        # ---- per 512-token group: transpose to [tok, 2E], combine halves,
        #      then routing math along the free dim ----
        for g in range(GPP):
            lg = work.tile([2 * E, 512], f32, tag="lg")
            nc.vector.tensor_copy(lg[:], psum_g[g][:])
            for c in range(4):
                t = tile0 + g * 4 + c
                ptp = pstp.tile([128, 2 * E], f32, tag="tp")
                nc.tensor.transpose(ptp[:], lg[:, c * 128:(c + 1) * 128],
                                    iden_sb[:])
                ltw = work.tile([128, 2 * E], f32, tag="ltw")
                nc.vector.tensor_copy(ltw[:], ptp[:])
                lt = work.tile([128, E], f32, tag="lt")
                nc.vector.tensor_tensor(lt[:], ltw[:, 0:E], ltw[:, E:2 * E],
                                        Alu.add)

                srt = work.tile([128, E], f32, tag="srt")
                nc.vector.max(srt[:], lt[:])
                sidx = work.tile([128, E], u32, tag="sidx")
                nc.vector.max_index(sidx[:], srt[:], lt[:])

                # aux partials: pacc += softmax(lt); cacc += top-2 one-hots
                prob = work.tile([128, E], f32, tag="prob")
                z = work.tile([128, 1], f32, tag="z")
                nc.scalar.activation(prob[:], lt[:], Act.Exp, accum_out=z[:])
                rz = work.tile([128, 1], f32, tag="rz")
                nc.vector.reciprocal(rz[:], z[:])
                nc.vector.scalar_tensor_tensor(pacc[:], prob[:], rz[:, 0:1],
                                               pacc[:], Alu.mult, Alu.add)
                nc.vector.scalar_tensor_tensor(cacc[:], lt[:], srt[:, 0:1],
                                               cacc[:], Alu.is_equal, Alu.add)
                nc.vector.scalar_tensor_tensor(cacc[:], lt[:], srt[:, 1:2],
                                               cacc[:], Alu.is_equal, Alu.add)

                # renormalized top-2 weights: w1 = 1/(1+e2), w2 = e2/(1+e2)
                d2 = work.tile([128, 1], f32, tag="d2")
                nc.vector.tensor_tensor(d2[:], srt[:, 1:2], srt[:, 0:1],
                                        Alu.subtract)
                e2 = work.tile([128, 1], f32, tag="e2")
                nc.scalar.activation(e2[:], d2[:], Act.Exp)
                t1 = work.tile([128, 1], f32, tag="t1")
                nc.vector.tensor_scalar_add(t1[:], e2[:], 1.0)
                w1 = w_out[:, 2 * t:2 * t + 1]
                nc.vector.reciprocal(w1, t1[:])
                nc.vector.tensor_tensor(w_out[:, 2 * t + 1:2 * t + 2], e2[:],
                                        w1, Alu.mult)
                nc.vector.tensor_copy(i_out[:, 2 * t:2 * t + 2], sidx[:, 0:2])

        # flush this phase's outputs while the next phase streams
        psl = slice(tile0, tile0 + GPP * 4)
        nc.sync.dma_start(
            rw.rearrange("(t p) c -> p t c", p=128)[:, psl, :],
            w_out[:].rearrange("p (t c) -> p t c", c=2)[:, psl, :])
        nc.sync.dma_start(
            xi.rearrange("(t p) c -> p t c", p=128)[:, psl, :],
            i_out[:].rearrange("p (t c) -> p t c", c=2)[:, psl, :])
        tok0 += TPP
        tile0 += GPP * 4

    nc.sync.dma_start(pacc_o[:, :], pacc[:])
    nc.sync.dma_start(cacc_o[:, :], cacc[:])


def _build():
    nc = bacc.Bacc("TRN2", target_bir_lowering=False, debug=False,
                   num_devices=N_CORES)
    hth = nc.dram_tensor("hth", [D, TPC], f16, kind="ExternalInput").ap()
    htl = nc.dram_tensor("htl", [D, TPC], f16, kind="ExternalInput").ap()
    gph = nc.dram_tensor("gph", [128, KC * 2 * E], f16,
                         kind="ExternalInput").ap()
    iden = nc.dram_tensor("iden", [2 * E, 2 * E], f32,
                          kind="ExternalInput").ap()
    rw = nc.dram_tensor("rw", [TPC, 2], f32, kind="ExternalOutput").ap()
    xi = nc.dram_tensor("xi", [TPC, 2], i32, kind="ExternalOutput").ap()
    pacc_o = nc.dram_tensor("pacc", [128, E], f32, kind="ExternalOutput").ap()
    cacc_o = nc.dram_tensor("cacc", [128, E], f32, kind="ExternalOutput").ap()

    with tile.TileContext(nc) as tc:
        with ExitStack() as ctx:
            _body(ctx, tc, hth, htl, gph, iden, rw, xi, pacc_o, cacc_o)
    nc.compile()
    return nc


_NC_CACHE = {}


def _get_nc():
    if "nc" not in _NC_CACHE:
        _NC_CACHE["nc"] = _build()
    return _NC_CACHE["nc"]


def _ensure_ntff_hook():
    """Register the axon NTFF profiling hook (the image's antenv lacks
    axon_hooks, so the boot-time registration degraded silently)."""
    import types

    if "antenv.axon_hooks" in sys.modules:
        return
    mod = types.ModuleType("antenv.axon_hooks")
    state = {"hook": None}
    mod.set_axon_ntff_profile_hook = lambda h: state.__setitem__("hook", h)
    mod.get_axon_ntff_profile_hook = lambda: state["hook"]
    sys.modules["antenv.axon_hooks"] = mod
    import antenv

    antenv.axon_hooks = mod
    try:
        from trn_agent_boot.trn_boot import _ntff_profile_via_ctypes

        hook = _ntff_profile_via_ctypes("/opt/axon/libaxon_pjrt.so")
        if hook is not None:
            mod.set_axon_ntff_profile_hook(hook)
    except Exception:
        pass
    # keep profiling artifacts local — no bucket in this container
    import concourse.bass_utils as bu

    bu.upload_artifacts = lambda tmpdir: tmpdir


def _pack_gate(gate_weight):
    """gate [E, D] f32 -> packed [128, KC*2E] fp16 with
    packed[p, k*2E + e]     = hi(g)(e, 128k+p)   (e < E)
    packed[p, k*2E + E + e] = lo(g)(e, 128k+p)."""
    gt = np.ascontiguousarray(gate_weight.T.astype(np.float32))     # [D, E]
    hi = gt.astype(np.float16)
    lo = (gt - hi.astype(np.float32)).astype(np.float16)
    both = np.concatenate([hi.reshape(KC, 128, E), lo.reshape(KC, 128, E)],
                          axis=2)                                   # [KC,128,2E]
    return np.ascontiguousarray(
        both.transpose(1, 0, 2).reshape(128, KC * 2 * E))


def _prep_inputs(hidden_states, gate_weight):
    hidden_states = np.asarray(hidden_states, dtype=np.float32)
    gate_weight = np.asarray(gate_weight, dtype=np.float32)
    gph = _pack_gate(gate_weight)
    iden = np.eye(2 * E, dtype=np.float32)
    in_maps = []
    for c in range(N_CORES):
        sh = np.ascontiguousarray(
            hidden_states[c * TPC:(c + 1) * TPC].T)                 # [D, TPC]
        hi = sh.astype(np.float16)
        lo = (sh - hi.astype(np.float32)).astype(np.float16)
        in_maps.append({"hth": hi, "htl": lo, "gph": gph, "iden": iden})
    return in_maps


def _finalize(results):
    rw = np.concatenate([r["rw"] for r in results], axis=0)
    xi = np.concatenate([r["xi"] for r in results], axis=0).astype(np.int32)
    psum = np.zeros(E, np.float64)
    csum = np.zeros(E, np.float64)
    for r in results:
        psum += r["pacc"].astype(np.float64).sum(axis=0)
        csum += r["cacc"].astype(np.float64).sum(axis=0)
    p_i = (psum / T).astype(np.float32)
    f_i = (csum / T).astype(np.float32)
    aux = np.float32(E) * np.dot(f_i, p_i)
    return rw.astype(np.float32), xi, np.float32(aux)


def run(hidden_states, gate_weight, trace=False, tmpdir=None):
    nc = _get_nc()
    if trace:
        _ensure_ntff_hook()
    in_maps = _prep_inputs(hidden_states, gate_weight)
    res = run_bass_kernel_spmd(nc, in_maps, list(range(N_CORES)), trace=trace,
                               tmpdir=tmpdir)
    return _finalize(res.results), res


def kernel(hidden_states, gate_weight):
    (rw, xi, aux), _ = run(hidden_states, gate_weight, trace=False)
    return rw, xi, aux


# revision 27
# speedup vs baseline: 1.3330x; 1.0212x over previous
"""MoE router (MixLora) Trainium2 kernel.

Computes, for hidden_states [16384, 4096] f32 and gate_weight [8, 4096] f32:
  logits = hidden @ gate.T            -> top-2 (values renormalized) + indices
  aux_loss = E * sum_i f_i * p_i      (load-balance loss)

Sharding: data-parallel over tokens across 8 NeuronCores (2048 tokens/core).
The gate weight is replicated. Aux-loss partials (per-expert softmax-prob sums
and top-2 counts) are computed per core and reduced on host during the gather.

Device strategy per core:
  - hidden shard is staged host-side as transposed fp16 hi/lo pair (ht ~ hi+lo)
    so the contraction dim D lands on SBUF partitions. Same HBM bytes as fp32.
  - logits.T accumulated in PSUM via 3 matmul passes (hi*ghi, hi*glo, lo*ghi)
    with the tiny packed gate as the stationary operand -> fp32-class accuracy
    at bf16-rate PE throughput.
  - logits.T [8, T] tiles are PE-transposed (identity trick) to [128, 8] so
    softmax / top-2 run along the free dim: DVE sort (max/max_index) gives the
    full descending order of the 8 experts; weights via w1 = 1/(1+exp(l2-l1)).
"""

import sys

if "/opt/trn_rl_repo" not in sys.path:
    sys.path.insert(0, "/opt/trn_rl_repo")

from contextlib import ExitStack

import numpy as np

import concourse.bass as bass
import concourse.tile as tile
from concourse import bacc, mybir
from concourse.bass_utils import run_bass_kernel_spmd

N_CORES = 8
T, D, E = 16384, 4096, 8
TPC = T // N_CORES            # tokens per core: 2048
KC = D // 128                 # contraction chunks: 32
NT = TPC // 128               # 128-token tiles per core: 16
# Token phases: uneven split so the final phase (whose matmuls + epilogue
# serialize after the DMA stream ends) is small. Per-phase KJ targets ~1 MiB
# DMA transfers (tile = [128, KJ, TPP] fp16).
PHASES = [(1536, 2), (512, 8)]        # (tokens, k-chunks per DMA)

f32 = mybir.dt.float32
f16 = mybir.dt.float16
i32 = mybir.dt.int32
u32 = mybir.dt.uint32
Alu = mybir.AluOpType
Act = mybir.ActivationFunctionType


def _body(ctx, tc, hth, htl, gph, iden, rw, xi, misc_o):
    nc = tc.nc

    const = ctx.enter_context(tc.tile_pool(name="const", bufs=1))
    hpool = ctx.enter_context(tc.tile_pool(name="hbuf", bufs=6))
    pstp = ctx.enter_context(tc.tile_pool(name="pstp", bufs=2, space="PSUM"))
    work = ctx.enter_context(tc.tile_pool(name="work", bufs=2))
    accp = ctx.enter_context(tc.tile_pool(name="accp", bufs=1))

    # const loads via SWDGE so the HWDGE rings start with the big streams
    gph_sb = const.tile([128, KC * 2 * E], f16)
    nc.gpsimd.dma_start(gph_sb[:], gph[:, :])
    iden_sb = const.tile([2 * E, 2 * E], f32)
    nc.gpsimd.dma_start(iden_sb[:], iden[:, :])

    # fin packs every end-of-kernel output into one DMA:
    # cols [0:8] last-phase weights, [8:16] last-phase indices (i32 bits),
    # [16:24] pacc, [24:32] cacc
    fin = accp.tile([128, 32], f32)
    fin_i = fin[:].bitcast(i32)
    pacc = fin[:, 16:24]
    nc.vector.memset(pacc, 0.0)
    cacc = fin[:, 24:32]
    nc.vector.memset(cacc, 0.0)
    w_out = accp.tile([128, 2 * NT], f32)
    i_out = accp.tile([128, 2 * NT], i32)

    # ---- logits.T: ps[0:8]  = (ghi . hh) + (ghi . hl)   (hl pass accumulates)
    #               ps[8:16] = (glo . hh)                 -> combined post-transpose
    psmm_ps = ctx.enter_context(tc.tile_pool(name="psbank", bufs=2, space="PSUM"))

    tok0 = 0
    tile0 = 0
    col0 = 0
    for ph, (TPP, KJ) in enumerate(PHASES):
        last_phase = ph == len(PHASES) - 1
        GPP = TPP // 512
        psum_g = [psmm_ps.tile([2 * E, 512], f32, name=f"ps{g}", tag=f"ps{g}")
                  for g in range(GPP)]
        for kj in range(KC // KJ):
            csl = slice(col0 + kj * KJ * TPP, col0 + (kj + 1) * KJ * TPP)
            hh = hpool.tile([128, KJ, TPP], f16, tag="hh")
            nc.sync.dma_start(
                hh[:], hth[:, csl].rearrange("p (j t) -> p j t", j=KJ))
            hl = hpool.tile([128, KJ, TPP], f16, tag="hl")
            nc.scalar.dma_start(
                hl[:], htl[:, csl].rearrange("p (j t) -> p j t", j=KJ))
            for j in range(KJ):
                k = kj * KJ + j
                last = k == KC - 1
                wsl = slice(k * 2 * E, (k + 1) * 2 * E)
                whsl = slice(k * 2 * E, k * 2 * E + E)

                def mm_hh(g, stop):
                    tsl = slice(g * 512, (g + 1) * 512)
                    nc.tensor.matmul(psum_g[g][:, :], gph_sb[:, wsl],
                                     hh[:, j, tsl], start=(k == 0), stop=stop)

                def mm_hl(g):
                    tsl = slice(g * 512, (g + 1) * 512)
                    nc.tensor.matmul(psum_g[g][0:E, :], gph_sb[:, whsl],
                                     hl[:, j, tsl], start=False, stop=False)

                if last:
                    # hl pass first so the final hh pass can close the
                    # accumulation group over the full [0:2E] region
                    for g in range(GPP):
                        mm_hl(g)
                    for g in range(GPP):
                        mm_hh(g, stop=True)
                else:
                    for g in range(GPP):
                        mm_hh(g, stop=False)
                    for g in range(GPP):
                        mm_hl(g)

        # ---- per 512-token group: transpose to [tok, 2E], combine halves,
        #      then routing math along the free dim ----
        for g in range(GPP):
            lg = work.tile([2 * E, 512], f32, tag="lg")
            nc.vector.tensor_copy(lg[:], psum_g[g][:])
            for c in range(4):
                t = tile0 + g * 4 + c
                ptp = pstp.tile([128, 2 * E], f32, tag="tp")
                nc.tensor.transpose(ptp[:], lg[:, c * 128:(c + 1) * 128],
                                    iden_sb[:])
                ltw = work.tile([128, 2 * E], f32, tag="ltw")
                nc.vector.tensor_copy(ltw[:], ptp[:])
                lt = work.tile([128, E], f32, tag="lt")
                nc.vector.tensor_tensor(lt[:], ltw[:, 0:E], ltw[:, E:2 * E],
                                        Alu.add)

                srt = work.tile([128, E], f32, tag="srt")
                nc.vector.max(srt[:], lt[:])
                sidx = work.tile([128, E], u32, tag="sidx")
                nc.vector.max_index(sidx[:], srt[:], lt[:])

                # aux partials: pacc += softmax(lt); cacc += top-2 one-hots
                prob = work.tile([128, E], f32, tag="prob")
                z = work.tile([128, 1], f32, tag="z")
                nc.scalar.activation(prob[:], lt[:], Act.Exp, accum_out=z[:])
                rz = work.tile([128, 1], f32, tag="rz")
                nc.vector.reciprocal(rz[:], z[:])
                nc.vector.scalar_tensor_tensor(pacc, prob[:], rz[:, 0:1],
                                               pacc, Alu.mult, Alu.add)
                nc.vector.scalar_tensor_tensor(cacc, lt[:], srt[:, 0:1],
                                               cacc, Alu.is_equal, Alu.add)
                nc.vector.scalar_tensor_tensor(cacc, lt[:], srt[:, 1:2],
                                               cacc, Alu.is_equal, Alu.add)

                # renormalized top-2 weights: w1 = 1/(1+e2), w2 = e2/(1+e2)
                d2 = work.tile([128, 1], f32, tag="d2")
                nc.vector.tensor_tensor(d2[:], srt[:, 1:2], srt[:, 0:1],
                                        Alu.subtract)
                e2 = work.tile([128, 1], f32, tag="e2")
                nc.scalar.activation(e2[:], d2[:], Act.Exp)
                t1 = work.tile([128, 1], f32, tag="t1")
                nc.vector.tensor_scalar_add(t1[:], e2[:], 1.0)
                if last_phase:
                    tl = g * 4 + c
                    wcol = fin[:, 2 * tl:2 * tl + 2]
                    icol = fin_i[:, 8 + 2 * tl:8 + 2 * tl + 2]
                else:
                    wcol = w_out[:, 2 * t:2 * t + 2]
                    icol = i_out[:, 2 * t:2 * t + 2]
                nc.vector.reciprocal(wcol[:, 0:1], t1[:])
                nc.vector.tensor_tensor(wcol[:, 1:2], e2[:], wcol[:, 0:1],
                                        Alu.mult)
                nc.vector.tensor_copy(icol, sidx[:, 0:2])

        if not last_phase:
            # flush this phase's outputs while the next phase streams
            psl = slice(tile0, tile0 + GPP * 4)
            nc.sync.dma_start(
                rw.rearrange("(t p) c -> p t c", p=128)[:, psl, :],
                w_out[:].rearrange("p (t c) -> p t c", c=2)[:, psl, :])
            nc.sync.dma_start(
                xi.rearrange("(t p) c -> p t c", p=128)[:, psl, :],
                i_out[:].rearrange("p (t c) -> p t c", c=2)[:, psl, :])
        tok0 += TPP
        tile0 += GPP * 4
        col0 += KC * TPP

    # single packed DMA for everything produced at the very end
    nc.sync.dma_start(misc_o[:, :], fin[:])


def _build():
    nc = bacc.Bacc("TRN2", target_bir_lowering=False, debug=False,
                   num_devices=N_CORES)
    img_cols = D * TPC // 128
    hth = nc.dram_tensor("hth", [128, img_cols], f16,
                         kind="ExternalInput").ap()
    htl = nc.dram_tensor("htl", [128, img_cols], f16,
                         kind="ExternalInput").ap()
    gph = nc.dram_tensor("gph", [128, KC * 2 * E], f16,
                         kind="ExternalInput").ap()
    iden = nc.dram_tensor("iden", [2 * E, 2 * E], f32,
                          kind="ExternalInput").ap()
    rw = nc.dram_tensor("rw", [TPC, 2], f32, kind="ExternalOutput").ap()
    xi = nc.dram_tensor("xi", [TPC, 2], i32, kind="ExternalOutput").ap()
    misc_o = nc.dram_tensor("misc", [128, 32], f32,
                            kind="ExternalOutput").ap()

    with tile.TileContext(nc) as tc:
        with ExitStack() as ctx:
            _body(ctx, tc, hth, htl, gph, iden, rw, xi, misc_o)
    nc.compile()
    return nc


_NC_CACHE = {}


def _get_nc():
    if "nc" not in _NC_CACHE:
        _NC_CACHE["nc"] = _build()
    return _NC_CACHE["nc"]


def _ensure_ntff_hook():
    """Register the axon NTFF profiling hook (the image's antenv lacks
    axon_hooks, so the boot-time registration degraded silently)."""
    import types

    if "antenv.axon_hooks" in sys.modules:
        return
    mod = types.ModuleType("antenv.axon_hooks")
    state = {"hook": None}
    mod.set_axon_ntff_profile_hook = lambda h: state.__setitem__("hook", h)
    mod.get_axon_ntff_profile_hook = lambda: state["hook"]
    sys.modules["antenv.axon_hooks"] = mod
    import antenv

    antenv.axon_hooks = mod
    try:
        from trn_agent_boot.trn_boot import _ntff_profile_via_ctypes

        hook = _ntff_profile_via_ctypes("/opt/axon/libaxon_pjrt.so")
        if hook is not None:
            mod.set_axon_ntff_profile_hook(hook)
    except Exception:
        pass
    # keep profiling artifacts local — no bucket in this container
    import concourse.bass_utils as bu

    bu.upload_artifacts = lambda tmpdir: tmpdir


def _pack_gate(gate_weight):
    """gate [E, D] f32 -> packed [128, KC*2E] fp16 with
    packed[p, k*2E + e]     = hi(g)(e, 128k+p)   (e < E)
    packed[p, k*2E + E + e] = lo(g)(e, 128k+p)."""
    gt = np.ascontiguousarray(gate_weight.T.astype(np.float32))     # [D, E]
    hi = gt.astype(np.float16)
    lo = (gt - hi.astype(np.float32)).astype(np.float16)
    both = np.concatenate([hi.reshape(KC, 128, E), lo.reshape(KC, 128, E)],
                          axis=2)                                   # [KC,128,2E]
    return np.ascontiguousarray(
        both.transpose(1, 0, 2).reshape(128, KC * 2 * E))


def _image(x):
    """[D, TPC] -> [128, D*TPC/128] laid out so each device DMA reads one
    contiguous per-partition chunk (exact SBUF tile images, in issue order)."""
    blocks = []
    tok0 = 0
    for TPP, KJ in PHASES:
        for kj in range(KC // KJ):
            blk = x[kj * KJ * 128:(kj + 1) * KJ * 128, tok0:tok0 + TPP]
            blocks.append(blk.reshape(KJ, 128, TPP).transpose(1, 0, 2)
                          .reshape(128, KJ * TPP))
        tok0 += TPP
    return np.ascontiguousarray(np.concatenate(blocks, axis=1))


def _prep_inputs(hidden_states, gate_weight):
    hidden_states = np.asarray(hidden_states, dtype=np.float32)
    gate_weight = np.asarray(gate_weight, dtype=np.float32)
    gph = _pack_gate(gate_weight)
    iden = np.eye(2 * E, dtype=np.float32)
    in_maps = []
    for c in range(N_CORES):
        sh = np.ascontiguousarray(
            hidden_states[c * TPC:(c + 1) * TPC].T)                 # [D, TPC]
        hi = sh.astype(np.float16)
        lo = (sh - hi.astype(np.float32)).astype(np.float16)
        in_maps.append({"hth": _image(hi), "htl": _image(lo), "gph": gph,
                        "iden": iden})
    return in_maps


def _finalize(results):
    t_last = PHASES[-1][0]                     # tokens in the packed phase
    nt_last = t_last // 128
    rws, xis = [], []
    psum = np.zeros(E, np.float64)
    csum = np.zeros(E, np.float64)
    for r in results:
        misc = r["misc"]
        rw_l = (misc[:, 0:2 * nt_last].reshape(128, nt_last, 2)
                .transpose(1, 0, 2).reshape(t_last, 2))
        xi_l = (misc[:, 8:8 + 2 * nt_last].view(np.int32)
                .reshape(128, nt_last, 2).transpose(1, 0, 2)
                .reshape(t_last, 2))
        rws.append(np.concatenate([r["rw"][:TPC - t_last], rw_l], axis=0))
        xis.append(np.concatenate([r["xi"][:TPC - t_last], xi_l], axis=0))
        psum += misc[:, 16:24].astype(np.float64).sum(axis=0)
        csum += misc[:, 24:32].astype(np.float64).sum(axis=0)
    rw = np.concatenate(rws, axis=0)
    xi = np.concatenate(xis, axis=0).astype(np.int32)
    p_i = (psum / T).astype(np.float32)
    f_i = (csum / T).astype(np.float32)
    aux = np.float32(E) * np.dot(f_i, p_i)
    return rw.astype(np.float32), xi, np.float32(aux)


def run(hidden_states, gate_weight, trace=False, tmpdir=None):
    nc = _get_nc()
    if trace:
        _ensure_ntff_hook()
    in_maps = _prep_inputs(hidden_states, gate_weight)
    res = run_bass_kernel_spmd(nc, in_maps, list(range(N_CORES)), trace=trace,
                               tmpdir=tmpdir)
    return _finalize(res.results), res


def kernel(hidden_states, gate_weight):
    (rw, xi, aux), _ = run(hidden_states, gate_weight, trace=False)
    return rw, xi, aux


# revision 29
# speedup vs baseline: 1.4297x; 1.0726x over previous
"""MoE router (MixLora) Trainium2 kernel.

Computes, for hidden_states [16384, 4096] f32 and gate_weight [8, 4096] f32:
  logits = hidden @ gate.T            -> top-2 (values renormalized) + indices
  aux_loss = E * sum_i f_i * p_i      (load-balance loss)

Sharding: data-parallel over tokens across 8 NeuronCores (2048 tokens/core).
The gate weight is replicated. Aux-loss partials (per-expert softmax-prob sums
and top-2 counts) are computed per core and reduced on host during the gather.

Device strategy per core:
  - hidden shard is staged host-side as transposed fp16 hi/lo pair (ht ~ hi+lo)
    so the contraction dim D lands on SBUF partitions. Same HBM bytes as fp32.
  - logits.T accumulated in PSUM via 3 matmul passes (hi*ghi, hi*glo, lo*ghi)
    with the tiny packed gate as the stationary operand -> fp32-class accuracy
    at bf16-rate PE throughput.
  - logits.T [8, T] tiles are PE-transposed (identity trick) to [128, 8] so
    softmax / top-2 run along the free dim: DVE sort (max/max_index) gives the
    full descending order of the 8 experts; weights via w1 = 1/(1+exp(l2-l1)).
"""

import sys

if "/opt/trn_rl_repo" not in sys.path:
    sys.path.insert(0, "/opt/trn_rl_repo")

from contextlib import ExitStack

import numpy as np

import concourse.bass as bass
import concourse.tile as tile
from concourse import bacc, mybir
from concourse.bass_utils import run_bass_kernel_spmd

N_CORES = 8
T, D, E = 16384, 4096, 8
TPC = T // N_CORES            # tokens per core: 2048
KC = D // 128                 # contraction chunks: 32
NT = TPC // 128               # 128-token tiles per core: 16
# Token phases: uneven split so the final phase (whose matmuls + epilogue
# serialize after the DMA stream ends) is small. Per-phase KJ targets ~1 MiB
# DMA transfers (tile = [128, KJ, TPP] fp16).
PHASES = [(1536, 2), (512, 8)]        # (tokens, k-chunks per DMA)

f32 = mybir.dt.float32
f16 = mybir.dt.float16
i32 = mybir.dt.int32
u32 = mybir.dt.uint32
Alu = mybir.AluOpType
Act = mybir.ActivationFunctionType


def _body(ctx, tc, hth, htl, gph, iden, rw, xi, misc_o):
    nc = tc.nc

    const = ctx.enter_context(tc.tile_pool(name="const", bufs=1))
    hpool = ctx.enter_context(tc.tile_pool(name="hbuf", bufs=6))
    pstp = ctx.enter_context(tc.tile_pool(name="pstp", bufs=2, space="PSUM"))
    work = ctx.enter_context(tc.tile_pool(name="work", bufs=2))
    accp = ctx.enter_context(tc.tile_pool(name="accp", bufs=1))

    # const loads via SWDGE so the HWDGE rings start with the big streams
    gph_sb = const.tile([128, KC * 2 * E], f16)
    nc.gpsimd.dma_start(gph_sb[:], gph[:, :])
    iden_sb = const.tile([2 * E, 2 * E], f32)
    nc.gpsimd.dma_start(iden_sb[:], iden[:, :])

    # fin packs every end-of-kernel output into one DMA:
    # cols [0:8] last-phase weights, [8:16] last-phase indices (i32 bits),
    # [16:24] pacc, [24:32] cacc
    fin = accp.tile([128, 32], f32)
    fin_i = fin[:].bitcast(i32)
    pacc = fin[:, 16:24]
    nc.vector.memset(pacc, 0.0)
    cacc = fin[:, 24:32]
    nc.vector.memset(cacc, 0.0)
    w_out = accp.tile([128, 2 * NT], f32)
    i_out = accp.tile([128, 2 * NT], i32)

    # ---- logits.T: ps[0:8]  = (ghi . hh) + (ghi . hl)   (hl pass accumulates)
    #               ps[8:16] = (glo . hh)                 -> combined post-transpose
    psmm_ps = ctx.enter_context(tc.tile_pool(name="psbank", bufs=2, space="PSUM"))

    # Emit every streaming DMA up front: SP carries only hh issues and ACT
    # only hl issues, so neither ring ever queues behind epilogue compute.
    # Pool slot-waits self-pace the stream against matmul consumption.
    stream = {}
    col0 = 0
    for ph, (TPP, KJ) in enumerate(PHASES):
        for kj in range(KC // KJ):
            csl = slice(col0 + kj * KJ * TPP, col0 + (kj + 1) * KJ * TPP)
            hh = hpool.tile([128, KJ, TPP], f16, tag="hh")
            nc.sync.dma_start(
                hh[:], hth[:, csl].rearrange("p (j t) -> p j t", j=KJ))
            hl = hpool.tile([128, KJ, TPP], f16, tag="hl")
            nc.scalar.dma_start(
                hl[:], htl[:, csl].rearrange("p (j t) -> p j t", j=KJ))
            stream[(ph, kj)] = (hh, hl)
        col0 += KC * TPP

    tok0 = 0
    tile0 = 0
    for ph, (TPP, KJ) in enumerate(PHASES):
        last_phase = ph == len(PHASES) - 1
        GPP = TPP // 512
        psum_g = [psmm_ps.tile([2 * E, 512], f32, name=f"ps{g}", tag=f"ps{g}")
                  for g in range(GPP)]
        for kj in range(KC // KJ):
            hh, hl = stream[(ph, kj)]
            for j in range(KJ):
                k = kj * KJ + j
                last = k == KC - 1
                wsl = slice(k * 2 * E, (k + 1) * 2 * E)
                whsl = slice(k * 2 * E, k * 2 * E + E)

                def mm_hh(g, stop):
                    tsl = slice(g * 512, (g + 1) * 512)
                    nc.tensor.matmul(psum_g[g][:, :], gph_sb[:, wsl],
                                     hh[:, j, tsl], start=(k == 0), stop=stop)

                def mm_hl(g):
                    tsl = slice(g * 512, (g + 1) * 512)
                    nc.tensor.matmul(psum_g[g][0:E, :], gph_sb[:, whsl],
                                     hl[:, j, tsl], start=False, stop=False)

                if last:
                    # hl pass first so the final hh pass can close the
                    # accumulation group over the full [0:2E] region
                    for g in range(GPP):
                        mm_hl(g)
                    for g in range(GPP):
                        mm_hh(g, stop=True)
                else:
                    for g in range(GPP):
                        mm_hh(g, stop=False)
                    for g in range(GPP):
                        mm_hl(g)

        # ---- per 512-token group: transpose to [tok, 2E], combine halves,
        #      then routing math along the free dim ----
        for g in range(GPP):
            lg = work.tile([2 * E, 512], f32, tag="lg")
            nc.vector.tensor_copy(lg[:], psum_g[g][:])
            for c in range(4):
                t = tile0 + g * 4 + c
                ptp = pstp.tile([128, 2 * E], f32, tag="tp")
                nc.tensor.transpose(ptp[:], lg[:, c * 128:(c + 1) * 128],
                                    iden_sb[:])
                ltw = work.tile([128, 2 * E], f32, tag="ltw")
                nc.vector.tensor_copy(ltw[:], ptp[:])
                lt = work.tile([128, E], f32, tag="lt")
                nc.vector.tensor_tensor(lt[:], ltw[:, 0:E], ltw[:, E:2 * E],
                                        Alu.add)

                srt = work.tile([128, E], f32, tag="srt")
                nc.vector.max(srt[:], lt[:])
                sidx = work.tile([128, E], u32, tag="sidx")
                nc.vector.max_index(sidx[:], srt[:], lt[:])

                # aux partials: pacc += softmax(lt); cacc += top-2 one-hots
                prob = work.tile([128, E], f32, tag="prob")
                z = work.tile([128, 1], f32, tag="z")
                nc.scalar.activation(prob[:], lt[:], Act.Exp, accum_out=z[:])
                rz = work.tile([128, 1], f32, tag="rz")
                nc.vector.reciprocal(rz[:], z[:])
                nc.vector.scalar_tensor_tensor(pacc, prob[:], rz[:, 0:1],
                                               pacc, Alu.mult, Alu.add)
                nc.vector.scalar_tensor_tensor(cacc, lt[:], srt[:, 0:1],
                                               cacc, Alu.is_equal, Alu.add)
                nc.vector.scalar_tensor_tensor(cacc, lt[:], srt[:, 1:2],
                                               cacc, Alu.is_equal, Alu.add)

                # renormalized top-2 weights: w1 = 1/(1+e2), w2 = e2/(1+e2)
                d2 = work.tile([128, 1], f32, tag="d2")
                nc.vector.tensor_tensor(d2[:], srt[:, 1:2], srt[:, 0:1],
                                        Alu.subtract)
                e2 = work.tile([128, 1], f32, tag="e2")
                nc.scalar.activation(e2[:], d2[:], Act.Exp)
                t1 = work.tile([128, 1], f32, tag="t1")
                nc.vector.tensor_scalar_add(t1[:], e2[:], 1.0)
                if last_phase:
                    tl = g * 4 + c
                    wcol = fin[:, 2 * tl:2 * tl + 2]
                    icol = fin_i[:, 8 + 2 * tl:8 + 2 * tl + 2]
                else:
                    wcol = w_out[:, 2 * t:2 * t + 2]
                    icol = i_out[:, 2 * t:2 * t + 2]
                nc.vector.reciprocal(wcol[:, 0:1], t1[:])
                nc.vector.tensor_tensor(wcol[:, 1:2], e2[:], wcol[:, 0:1],
                                        Alu.mult)
                nc.vector.tensor_copy(icol, sidx[:, 0:2])

        if not last_phase:
            # flush this phase's outputs (SWDGE ring: keeps SP/ACT clear)
            psl = slice(tile0, tile0 + GPP * 4)
            nc.gpsimd.dma_start(
                rw.rearrange("(t p) c -> p t c", p=128)[:, psl, :],
                w_out[:].rearrange("p (t c) -> p t c", c=2)[:, psl, :])
            nc.gpsimd.dma_start(
                xi.rearrange("(t p) c -> p t c", p=128)[:, psl, :],
                i_out[:].rearrange("p (t c) -> p t c", c=2)[:, psl, :])
        tok0 += TPP
        tile0 += GPP * 4

    # single packed DMA for everything produced at the very end
    nc.gpsimd.dma_start(misc_o[:, :], fin[:])


def _build():
    nc = bacc.Bacc("TRN2", target_bir_lowering=False, debug=False,
                   num_devices=N_CORES)
    img_cols = D * TPC // 128
    hth = nc.dram_tensor("hth", [128, img_cols], f16,
                         kind="ExternalInput").ap()
    htl = nc.dram_tensor("htl", [128, img_cols], f16,
                         kind="ExternalInput").ap()
    gph = nc.dram_tensor("gph", [128, KC * 2 * E], f16,
                         kind="ExternalInput").ap()
    iden = nc.dram_tensor("iden", [2 * E, 2 * E], f32,
                          kind="ExternalInput").ap()
    rw = nc.dram_tensor("rw", [TPC, 2], f32, kind="ExternalOutput").ap()
    xi = nc.dram_tensor("xi", [TPC, 2], i32, kind="ExternalOutput").ap()
    misc_o = nc.dram_tensor("misc", [128, 32], f32,
                            kind="ExternalOutput").ap()

    with tile.TileContext(nc) as tc:
        with ExitStack() as ctx:
            _body(ctx, tc, hth, htl, gph, iden, rw, xi, misc_o)
    nc.compile()
    return nc


_NC_CACHE = {}


def _get_nc():
    if "nc" not in _NC_CACHE:
        _NC_CACHE["nc"] = _build()
    return _NC_CACHE["nc"]


def _ensure_ntff_hook():
    """Register the axon NTFF profiling hook (the image's antenv lacks
    axon_hooks, so the boot-time registration degraded silently)."""
    import types

    if "antenv.axon_hooks" in sys.modules:
        return
    mod = types.ModuleType("antenv.axon_hooks")
    state = {"hook": None}
    mod.set_axon_ntff_profile_hook = lambda h: state.__setitem__("hook", h)
    mod.get_axon_ntff_profile_hook = lambda: state["hook"]
    sys.modules["antenv.axon_hooks"] = mod
    import antenv

    antenv.axon_hooks = mod
    try:
        from trn_agent_boot.trn_boot import _ntff_profile_via_ctypes

        hook = _ntff_profile_via_ctypes("/opt/axon/libaxon_pjrt.so")
        if hook is not None:
            mod.set_axon_ntff_profile_hook(hook)
    except Exception:
        pass
    # keep profiling artifacts local — no bucket in this container
    import concourse.bass_utils as bu

    bu.upload_artifacts = lambda tmpdir: tmpdir


def _pack_gate(gate_weight):
    """gate [E, D] f32 -> packed [128, KC*2E] fp16 with
    packed[p, k*2E + e]     = hi(g)(e, 128k+p)   (e < E)
    packed[p, k*2E + E + e] = lo(g)(e, 128k+p)."""
    gt = np.ascontiguousarray(gate_weight.T.astype(np.float32))     # [D, E]
    hi = gt.astype(np.float16)
    lo = (gt - hi.astype(np.float32)).astype(np.float16)
    both = np.concatenate([hi.reshape(KC, 128, E), lo.reshape(KC, 128, E)],
                          axis=2)                                   # [KC,128,2E]
    return np.ascontiguousarray(
        both.transpose(1, 0, 2).reshape(128, KC * 2 * E))


def _image(x):
    """[D, TPC] -> [128, D*TPC/128] laid out so each device DMA reads one
    contiguous per-partition chunk (exact SBUF tile images, in issue order)."""
    blocks = []
    tok0 = 0
    for TPP, KJ in PHASES:
        for kj in range(KC // KJ):
            blk = x[kj * KJ * 128:(kj + 1) * KJ * 128, tok0:tok0 + TPP]
            blocks.append(blk.reshape(KJ, 128, TPP).transpose(1, 0, 2)
                          .reshape(128, KJ * TPP))
        tok0 += TPP
    return np.ascontiguousarray(np.concatenate(blocks, axis=1))


def _prep_inputs(hidden_states, gate_weight):
    hidden_states = np.asarray(hidden_states, dtype=np.float32)
    gate_weight = np.asarray(gate_weight, dtype=np.float32)
    gph = _pack_gate(gate_weight)
    iden = np.eye(2 * E, dtype=np.float32)
    in_maps = []
    for c in range(N_CORES):
        sh = np.ascontiguousarray(
            hidden_states[c * TPC:(c + 1) * TPC].T)                 # [D, TPC]
        hi = sh.astype(np.float16)
        lo = (sh - hi.astype(np.float32)).astype(np.float16)
        in_maps.append({"hth": _image(hi), "htl": _image(lo), "gph": gph,
                        "iden": iden})
    return in_maps


def _finalize(results):
    t_last = PHASES[-1][0]                     # tokens in the packed phase
    nt_last = t_last // 128
    rws, xis = [], []
    psum = np.zeros(E, np.float64)
    csum = np.zeros(E, np.float64)
    for r in results:
        misc = r["misc"]
        rw_l = (misc[:, 0:2 * nt_last].reshape(128, nt_last, 2)
                .transpose(1, 0, 2).reshape(t_last, 2))
        xi_l = (misc[:, 8:8 + 2 * nt_last].view(np.int32)
                .reshape(128, nt_last, 2).transpose(1, 0, 2)
                .reshape(t_last, 2))
        rws.append(np.concatenate([r["rw"][:TPC - t_last], rw_l], axis=0))
        xis.append(np.concatenate([r["xi"][:TPC - t_last], xi_l], axis=0))
        psum += misc[:, 16:24].astype(np.float64).sum(axis=0)
        csum += misc[:, 24:32].astype(np.float64).sum(axis=0)
    rw = np.concatenate(rws, axis=0)
    xi = np.concatenate(xis, axis=0).astype(np.int32)
    p_i = (psum / T).astype(np.float32)
    f_i = (csum / T).astype(np.float32)
    aux = np.float32(E) * np.dot(f_i, p_i)
    return rw.astype(np.float32), xi, np.float32(aux)


def run(hidden_states, gate_weight, trace=False, tmpdir=None):
    nc = _get_nc()
    if trace:
        _ensure_ntff_hook()
    in_maps = _prep_inputs(hidden_states, gate_weight)
    res = run_bass_kernel_spmd(nc, in_maps, list(range(N_CORES)), trace=trace,
                               tmpdir=tmpdir)
    return _finalize(res.results), res


def kernel(hidden_states, gate_weight):
    (rw, xi, aux), _ = run(hidden_states, gate_weight, trace=False)
    return rw, xi, aux
